# revision 24
# baseline (speedup 1.0000x reference)
"""Trainium2 Bass kernel for DAS (delay-and-sum) ultrasound beamforming.

Math: the per-(t,e,z) delay/phase/apodization depend on (t,e) only through
vx = gx[t]-ex[e], i.e. on delta = t-e (Toeplitz geometry). Per-delta tables
(gather index, fused interp/rotation/apod weights) are computed on host from
the small geometry inputs; the sample data is processed on 8 NeuronCores.

The wall clock is dominated by host->device transfer over the axon tunnel
(~55-75 MB/s), so the packing minimizes bytes three ways:
  1. delta trim: the dynamic-aperture mask kills every delta with |delta|>=100
     (and, per delta, all z below a threshold), so only 199 of 255 diagonals
     ship, each with only its contributing z-blocks.
  2. sample-window trim: per delta only samples in [i0(zlo), i0(zmax)+1] are
     ever gathered (~1.2-1.4k of 4096), so only that window ships.
  3. int8 quantization (clip at 4 sigma, scale folded into the weight tables;
     ~0.9% rel error vs the 2e-2 budget). I/Q sample pairs are packed as 4
     int8 bytes = one f32 "container" so gpsimd ap_gather (which needs
     4-byte units) fetches I[2c],Q[2c],I[2c+1],Q[2c+1] in one d=1 lookup;
     interpolation parity (i0 even/odd) is folded into the host-built
     weight tables.

Device per (core, slot) = one delta diagonal: DMA the diagonal's container
rows -> ap_gather at container c and c+1 (indices shared across partitions =
transmits) -> extract 6 int8 byte-planes to f32 -> PE transpose to [z, t] ->
multiply by per-delta weight columns (free-axis broadcast) and accumulate.
Host sums the 8 per-core fp16 partial [z,t] accumulators.

The apodization mask is validated exactly per (t,e,z) on host; any mismatch
vs the delta-representative mask is fixed with sparse host corrections
(zero for the reference geometry).

SPMD uniformity: 199 active deltas + 1 dummy = 200 (core,slot) instances in
25 slots x 8 cores, grouped by |delta| so every core's slot k has the same
compiled extents/offsets.
"""
import os
import sys

for _p in ('/opt/trn_rl_repo', '/root/.axon_site/_ro/trn_rl_repo'):
    if os.path.isdir(_p) and _p not in sys.path:
        sys.path.append(_p)

import numpy as np


def _enable_jax_compile_cache():
    # run_bass_kernel_spmd re-jits (and re-runs the BIR->NEFF pipeline) on
    # every call; the persistent cache turns that ~0.4s into a disk hit.
    try:
        import jax
        jax.config.update("jax_compilation_cache_dir", "/tmp/jaxcache")
        jax.config.update("jax_persistent_cache_min_entry_size_bytes", 0)
        jax.config.update("jax_persistent_cache_min_compile_time_secs", 0.0)
    except Exception:
        pass


_enable_jax_compile_cache()


def _install_pjrt_jit_cache():
    """Memoize the jax plumbing of bass2jax.run_bass_via_pjrt per Bass module.

    run_bass_kernel_spmd builds a fresh jit(shard_map(...)) closure on every
    call, so each call pays ~80ms of re-tracing/lowering for an identical
    computation. This caches the jitted callable per `nc`; inputs are still
    uploaded and the NEFF still executes on every call.
    """
    try:
        from concourse import bass2jax, mybir
        import jax
        from jax.sharding import Mesh, PartitionSpec
        import warnings
        with warnings.catch_warnings():
            warnings.simplefilter("ignore")
            from jax.experimental.shard_map import shard_map
    except Exception:
        return
    if getattr(bass2jax.run_bass_via_pjrt, '_das_cached', False):
        return
    orig = bass2jax.run_bass_via_pjrt
    cache = {}

    def cached_run(nc, in_maps, n_cores):
        if n_cores == 1 or nc.dbg_addr is not None:
            return orig(nc, in_maps, n_cores)
        ent = cache.get(id(nc))
        if ent is None:
            bass2jax.install_neuronx_cc_hook()
            partition_name = (nc.partition_id_tensor.name
                              if nc.partition_id_tensor else None)
            in_names, out_names, out_avals, zero_shapes = [], [], [], []
            for alloc in nc.m.functions[0].allocations:
                if not isinstance(alloc, mybir.MemoryLocationSet):
                    continue
                name = alloc.memorylocations[0].name
                if alloc.kind == "ExternalInput":
                    if name != partition_name:
                        in_names.append(name)
                elif alloc.kind == "ExternalOutput":
                    shape = tuple(alloc.tensor_shape)
                    dtype = mybir.dt.np(alloc.dtype)
                    out_avals.append(jax.core.ShapedArray(shape, dtype))
                    out_names.append(name)
                    zero_shapes.append((shape, dtype))
            n_params = len(in_names)
            all_names = in_names + out_names + (
                [partition_name] if partition_name else [])
            donate = tuple(range(n_params, n_params + len(out_names)))

            def _body(*args):
                operands = list(args)
                if partition_name is not None:
                    operands.append(bass2jax.partition_id_tensor())
                return tuple(_bind(*operands))

            def _bind(*operands):
                return bass2jax._bass_exec_p.bind(
                    *operands, out_avals=tuple(out_avals),
                    in_names=tuple(all_names), out_names=tuple(out_names),
                    lowering_input_output_aliases=(),
                    sim_require_finite=True, sim_require_nnan=True, nc=nc)

            devices = jax.devices()[:n_cores]
            mesh = Mesh(np.asarray(devices), ("core",))
            sharding = jax.sharding.NamedSharding(mesh, PartitionSpec("core"))
            nio = n_params + len(out_names)
            sharded = jax.jit(
                shard_map(_body, mesh=mesh,
                          in_specs=(PartitionSpec("core"),) * nio,
                          out_specs=(PartitionSpec("core"),) * len(out_names),
                          check_rep=False),
                donate_argnums=donate, keep_unused=True)
            ent = (in_names, out_names, out_avals, zero_shapes, sharded,
                   devices, sharding)
            cache[id(nc)] = ent
        (in_names, out_names, out_avals, zero_shapes, sharded,
         devices, sharding) = ent

        def _global(per_core):
            # assemble a sharded global array from per-device shards without
            # the concat + re-slice host memcpy round-trip
            shape = (n_cores * per_core[0].shape[0], *per_core[0].shape[1:])
            shards = [jax.device_put(a, dev)
                      for a, dev in zip(per_core, devices)]
            return jax.make_array_from_single_device_arrays(
                shape, sharding, shards)

        global_in = [_global([np.asarray(m[name]) for m in in_maps])
                     for name in in_names]
        # donated output buffers: allocate+zero on device, skip the upload
        zkey = (id(nc), 'zeros')
        zfn = cache.get(zkey)
        if zfn is None:
            import jax.numpy as jnp
            zfn = jax.jit(
                lambda: tuple(
                    jnp.zeros((n_cores * shape[0], *shape[1:]), dtype)
                    for (shape, dtype) in zero_shapes),
                out_shardings=(sharding,) * len(zero_shapes))
            cache[zkey] = zfn
        out_arrs = sharded(*global_in, *zfn())
        return [
            {name: np.asarray(out_arrs[i]).reshape(
                n_cores, *out_avals[i].shape)[c]
             for i, name in enumerate(out_names)}
            for c in range(n_cores)]

    cached_run._das_cached = True
    bass2jax.run_bass_via_pjrt = cached_run


_install_pjrt_jit_cache()

T, E, S, Z = 128, 128, 4096, 2048
PI = 3.14159265359
MIN_WIDTH = 0.001
N_CORES = 8
DUMMY = 999
QCLIP = 4.0
QSCALE = np.float32(QCLIP / 127.0)   # dequant scale, folded into weights


def _f32(x):
    return np.asarray(x, dtype=np.float32)


# ---------------------------------------------------------------- host math
def compute_tables(grid, tx_ori, ele_pos, time_zero, fs, c, fdemod, rxfnum):
    grid = _f32(grid); tx_ori = _f32(tx_ori); ele_pos = _f32(ele_pos)
    time_zero = _f32(time_zero)
    gx = grid[:, 0, 0]
    zax = grid[0, :, 2]
    ex = ele_pos[:, 0]

    vx_te = (gx[:, None] - ex[None, :]).astype(np.float32)
    vz = zax.astype(np.float32)
    with np.errstate(divide='ignore', invalid='ignore'):
        ratio = np.abs(vz[None, None, :] / vx_te[:, :, None])
    m = ratio > np.float32(rxfnum)
    m |= (np.abs(vx_te) <= np.float32(MIN_WIDTH))[:, :, None]
    m |= ((vx_te >= np.float32(MIN_WIDTH)) & (gx[:, None] <= ex[0]))[:, :, None]
    m |= ((vx_te <= np.float32(-MIN_WIDTH)) & (gx[:, None] >= ex[-1]))[:, :, None]
    mask_exact = m

    d3 = grid - tx_ori[:, None, :]
    txdel = np.sqrt((d3 * d3).sum(-1, dtype=np.float32)).astype(np.float32)

    nd = 255
    i0_tab = np.zeros((nd, Z), np.int32)
    frac_tab = np.zeros((nd, Z), np.float32)
    ct_tab = np.zeros((nd, Z), np.float32)
    st_tab = np.zeros((nd, Z), np.float32)
    v0_tab = np.zeros((nd, Z), np.float32)
    v1_tab = np.zeros((nd, Z), np.float32)
    mask_tab = np.zeros((nd, Z), bool)
    for delta in range(-127, 128):
        t_rep = max(0, delta); e_rep = t_rep - delta
        vx = vx_te[t_rep, e_rep]
        rx = np.sqrt(vx * vx + vz * vz).astype(np.float32)
        delays = ((txdel[t_rep] + rx) / np.float32(c)
                  + time_zero[t_rep]) * np.float32(fs)
        i0f = np.floor(delays)
        frac = (delays - i0f).astype(np.float32)
        i0 = i0f.astype(np.int32)
        tshift = delays / np.float32(fs) - zax * np.float32(2.0) / np.float32(c)
        theta = (np.float32(2.0 * PI * fdemod) * tshift).astype(np.float32)
        j = delta + 127
        i0_tab[j] = i0
        frac_tab[j] = frac
        ct_tab[j] = np.cos(theta, dtype=np.float32)
        st_tab[j] = np.sin(theta, dtype=np.float32)
        v0_tab[j] = (i0 >= 0) & (i0 < S)
        v1_tab[j] = (i0 + 1 >= 0) & (i0 + 1 < S)
        mask_tab[j] = mask_exact[t_rep, e_rep]
    return dict(i0=i0_tab, frac=frac_tab, ct=ct_tab, st=st_tab,
                v0=v0_tab, v1=v1_tab, mask_tab=mask_tab,
                mask_exact=mask_exact)


def build_slot_assignment(tabs):
    """25 slots x 8 cores over the mask-active deltas, grouped by |delta|.

    Per slot (uniform across cores, compile-time): toff/ext (transmit rows),
    blk0/nblk (contributing z-blocks of 128), ncont (container window width).
    Per (slot, core): the delta, and its container base c0.
    """
    i0 = tabs['i0']; mask = tabs['mask_tab']
    active = sorted(d for d in range(-127, 128) if mask[d + 127].any())
    dmax = max(abs(d) for d in active)
    assert all(abs(d) <= dmax for d in active)

    zlo = {}
    c0 = {}
    cmax = {}
    for d in active:
        j = d + 127
        nz = np.nonzero(mask[j])[0]
        zlo[d] = int(nz[0])
        assert mask[j, zlo[d]:].all(), "mask must be a suffix in z"
        assert (np.diff(i0[j]) >= 0).all(), "i0 must be monotone in z"
        lo = int(i0[j, zlo[d]]); hi = int(i0[j, Z - 1])
        assert lo >= 0 and hi + 1 < S, "window must be interior"
        c0[d] = lo >> 1
        cmax[d] = (hi >> 1) + 1          # container of highest sample, +1 pad

    # groups: positive runs of 8 ascending from 4, negative mirror, plus the
    # near-diagonal mixed group {0,+-1,+-2,+-3} + dummy
    small = [d for d in active if abs(d) <= 3]
    pos = sorted(d for d in active if d > 3)
    neg = sorted((d for d in active if d < -3), key=lambda d: -d)
    assert len(small) == 7 and len(pos) % 8 == 0 and len(neg) % 8 == 0

    groups = [small + [DUMMY]]
    for i in range(0, len(pos), 8):
        groups.append(pos[i:i + 8])
    for i in range(0, len(neg), 8):
        groups.append(neg[i:i + 8])

    slots = []
    coff = ioff = woff = 0
    for g in groups:
        real = [d for d in g if d != DUMMY]
        toff = min(max(0, d) for d in real)
        ext = 128 - min(abs(d) for d in real)
        blk0 = min(zlo[d] for d in real) // 128
        nblk = 16 - blk0
        ncont = max(cmax[d] - c0[d] + 1 for d in real)
        slots.append(dict(deltas=list(g), toff=toff, ext=ext, blk0=blk0,
                          nblk=nblk, ncont=ncont,
                          c0={d: c0[d] for d in real},
                          coff=coff, ioff=ioff, woff=woff))
        coff += ext * ncont
        ioff += nblk * 8
        woff += 4 * nblk
    return dict(slots=slots, tot_cont=coff, tot_idx=ioff, tot_w=woff)


def build_weight_tables(tabs):
    """Per-delta z-tables for the 6 gathered byte planes.

    Gather0 fetches container c=(i0>>1)-c0: bytes (I[2c],Q[2c],I[2c+1],Q[2c+1]);
    gather1 fetches container c+1. With parity p = i0&1:
      I[i0]   = p==0 ? g0.b0 : g0.b2      I[i0+1] = p==0 ? g0.b2 : g1.b0
    Plane coefficients a = (a_b0, a_b2, a_g1b0):
      a_b0 = even*(1-frac); a_b2 = even*frac + odd*(1-frac); a_g1b0 = odd*frac
    Shipped tables (x apod x quant scale): C = ct, S = st, plus a0 and a2;
    the device expands u_j = C*a_j, v_j = S*a_j with a1 = 1 - a0 - a2.
      accI = sum_j u_j*Iplane_j - v_j*Qplane_j
      accQ = sum_j v_j*Iplane_j + u_j*Qplane_j
    """
    apod = tabs['mask_tab'].astype(np.float32) * QSCALE
    frac = tabs['frac']
    even = ((tabs['i0'] & 1) == 0).astype(np.float32)
    odd = np.float32(1.0) - even
    out = np.empty((255, 4, Z), np.float32)
    out[:, 0] = apod * tabs['ct']
    out[:, 1] = apod * tabs['st']
    out[:, 2] = even * (np.float32(1.0) - frac)
    out[:, 3] = odd * frac
    return out


def corrections(idata, qdata, tabs):
    corrI = np.zeros((T, Z), np.float32)
    corrQ = np.zeros((T, Z), np.float32)
    i0c = np.clip(tabs['i0'], 0, S - 1)
    i1c = np.clip(tabs['i0'] + 1, 0, S - 1)
    for delta in range(-127, 128):
        j = delta + 127
        ts = np.arange(max(0, delta), min(T - 1, T - 1 + delta) + 1)
        es = ts - delta
        dm = (tabs['mask_exact'][ts, es, :].astype(np.int8)
              - tabs['mask_tab'][j][None, :].astype(np.int8))
        nz = np.argwhere(dm != 0)
        if nz.size == 0:
            continue
        ti, zi = nz[:, 0], nz[:, 1]
        tt, ee = ts[ti], es[ti]
        sgn = dm[ti, zi].astype(np.float32)
        f = tabs['frac'][j][zi]; ct = tabs['ct'][j][zi]; st = tabs['st'][j][zi]
        v0 = tabs['v0'][j][zi]; v1 = tabs['v1'][j][zi]
        I0 = idata[tt, ee, i0c[j][zi]] * v0; I1 = idata[tt, ee, i1c[j][zi]] * v1
        Q0 = qdata[tt, ee, i0c[j][zi]] * v0; Q1 = qdata[tt, ee, i1c[j][zi]] * v1
        fi = (1 - f) * I0 + f * I1
        fq = (1 - f) * Q0 + f * Q1
        np.add.at(corrI, (tt, zi), sgn * (ct * fi - st * fq))
        np.add.at(corrQ, (tt, zi), sgn * (ct * fq + st * fi))
    return corrI, corrQ


# ------------------------------------------------------------- bass program
_CACHE = {}


def _build_program(meta):
    import concourse.bacc as bacc
    import concourse.mybir as mybir
    from concourse.tile import TileContext
    from concourse.masks import make_identity

    F32 = mybir.dt.float32
    F16 = mybir.dt.float16
    I16 = mybir.dt.int16
    I8 = mybir.dt.int8
    slots = meta['slots']

    nc = bacc.Bacc("TRN2", target_bir_lowering=False, debug=False,
                   num_devices=N_CORES)
    rows_d = nc.dram_tensor("rows", [meta['tot_cont']], F32,
                            kind="ExternalInput").ap()
    # wrapped gather indices repeat identically across the 8 16-partition
    # groups, so only 16 partitions ship; a broadcast DMA replicates x8.
    idx_d = nc.dram_tensor("idx", [16, meta['tot_idx']], I16,
                           kind="ExternalInput").ap()
    wts_d = nc.dram_tensor("wts", [128, meta['tot_w']], F16,
                           kind="ExternalInput").ap()
    # combined I|Q partial sums, reduce-scattered across the 8 cores:
    # core c outputs partitions [16c, 16c+16) of the summed [128, 2Z].
    acc_d = nc.dram_tensor("acc", [16, 2 * Z], F16, kind="ExternalOutput").ap()

    with TileContext(nc) as tc:
        with tc.tile_pool(name="data", bufs=2) as dpool, \
             tc.tile_pool(name="gout", bufs=2) as gpool, \
             tc.tile_pool(name="plane", bufs=2) as plpool, \
             tc.tile_pool(name="small", bufs=2) as spool, \
             tc.tile_pool(name="tmp", bufs=3) as tpool, \
             tc.tile_pool(name="accp", bufs=1) as apool, \
             tc.tile_pool(name="dram", bufs=1, space="DRAM") as drpool, \
             tc.tile_pool(name="psum", bufs=2, space="PSUM") as ppool:
            ident = apool.tile([128, 128], F32, tag="ident")
            make_identity(nc, ident[:])
            accI = apool.tile([128, 16, 128], F32, tag="accI")
            accQ = apool.tile([128, 16, 128], F32, tag="accQ")
            nc.vector.memset(accI[:], 0.0)
            nc.vector.memset(accQ[:], 0.0)

            for k, sl in enumerate(slots):
                ext, toff = sl['ext'], sl['toff']
                nblk, ncont = sl['nblk'], sl['ncont']
                nidx = nblk * 128
                data_t = dpool.tile([128, ncont], F32, tag="data")
                nc.sync.dma_start(
                    out=data_t[0:ext],
                    in_=rows_d[sl['coff']:sl['coff'] + ext * ncont]
                    .rearrange("(r c) -> r c", c=ncont))
                idx_t = spool.tile([128, nblk * 8], I16, tag="idx")
                for g in range(8):
                    nc.sync.dma_start(
                        out=idx_t[16 * g:16 * g + 16, :],
                        in_=idx_d[:, sl['ioff']:sl['ioff'] + nblk * 8])
                idx1_t = spool.tile([128, nblk * 8], I16, tag="idx1")
                nc.any.tensor_scalar(out=idx1_t[:], in0=idx_t[:], scalar1=1,
                                     scalar2=None, op0=mybir.AluOpType.add)
                w_t = spool.tile([128, 4 * nblk], F16, tag="wts")
                nc.sync.dma_start(
                    out=w_t[:], in_=wts_d[:, sl['woff']:sl['woff'] + 4 * nblk])
                # expand (C, S, a0, a2) -> [u0|u1|u2|v0|v1|v2], a1 = 1-a0-a2
                wf = spool.tile([128, 6 * nblk], F16, tag="wfull")
                C, Sw = w_t[:, 0:nblk], w_t[:, nblk:2 * nblk]
                A0, A2 = w_t[:, 2 * nblk:3 * nblk], w_t[:, 3 * nblk:4 * nblk]
                for base, src in ((0, C), (3, Sw)):
                    s0 = wf[:, base * nblk:(base + 1) * nblk]
                    s1 = wf[:, (base + 1) * nblk:(base + 2) * nblk]
                    s2 = wf[:, (base + 2) * nblk:(base + 3) * nblk]
                    nc.any.tensor_tensor(out=s0, in0=src, in1=A0,
                                         op=mybir.AluOpType.mult)
                    nc.any.tensor_tensor(out=s2, in0=src, in1=A2,
                                         op=mybir.AluOpType.mult)
                    nc.any.tensor_tensor(out=s1, in0=src, in1=s0,
                                         op=mybir.AluOpType.subtract)
                    nc.any.tensor_tensor(out=s1, in0=s1, in1=s2,
                                         op=mybir.AluOpType.subtract)

                gout0 = gpool.tile([128, nidx], F32, tag="g0")
                gout1 = gpool.tile([128, nidx], F32, tag="g1")
                ch = min(128, (ext + 15) // 16 * 16)
                nc.gpsimd.ap_gather(gout0[0:ch], data_t[0:ch], idx_t[:],
                                    channels=ch, num_elems=ncont, d=1,
                                    num_idxs=nidx)
                nc.gpsimd.ap_gather(gout1[0:ch], data_t[0:ch], idx1_t[:],
                                    channels=ch, num_elems=ncont, d=1,
                                    num_idxs=nidx)
                g0b = gout0[:].bitcast(I8).rearrange("p (n b) -> p n b", b=4)
                g1b = gout1[:].bitcast(I8).rearrange("p (n b) -> p n b", b=4)

                # (byte-plane source, weight-table index j, is_I_plane)
                for (src, j, isI) in ((g0b[:, :, 0], 0, True),
                                      (g0b[:, :, 2], 1, True),
                                      (g1b[:, :, 0], 2, True),
                                      (g0b[:, :, 1], 0, False),
                                      (g0b[:, :, 3], 1, False),
                                      (g1b[:, :, 1], 2, False)):
                    plane = plpool.tile([128, nidx], F32, tag="plane")
                    nc.any.tensor_copy(out=plane[0:ch], in_=src[0:ch])
                    big = ppool.tile([128, nblk, 128], F32, space="PSUM",
                                     tag="big")
                    for blk in range(nblk):
                        nc.tensor.transpose(
                            out=big[:, blk, 0:ch],
                            in_=plane[0:ch, blk * 128:(blk + 1) * 128],
                            identity=ident[0:ch, 0:ch])
                    u = wf[:, j * nblk:(j + 1) * nblk]
                    v = wf[:, (3 + j) * nblk:(4 + j) * nblk]
                    pairs = (((accI, u, mybir.AluOpType.add),
                              (accQ, v, mybir.AluOpType.add)) if isI else
                             ((accI, v, mybir.AluOpType.subtract),
                              (accQ, u, mybir.AluOpType.add)))
                    for (acc, w_ap, op) in pairs:
                        tmp = tpool.tile([128, nblk, 128], F32, tag="tmp")
                        nc.any.tensor_tensor(
                            out=tmp[:, :, 0:ext], in0=big[:, :, 0:ext],
                            in1=w_ap.broadcast_to([128, nblk, ext]),
                            op=mybir.AluOpType.mult)
                        acc_ap = acc[:, sl['blk0']:sl['blk0'] + nblk,
                                     toff:toff + ext]
                        nc.any.tensor_tensor(out=acc_ap, in0=acc_ap,
                                             in1=tmp[:, :, 0:ext], op=op)

            out16 = apool.tile([128, 2, 16, 128], F16, tag="o16")
            nc.any.tensor_copy(out=out16[:, 0], in_=accI[:])
            nc.any.tensor_copy(out=out16[:, 1], in_=accQ[:])
            rs_in = drpool.tile([128, 2 * Z], F16, tag="rs_in")
            rs_out = drpool.tile([16, 2 * Z], F16, tag="rs_out")
            nc.sync.dma_start(out=rs_in[:],
                              in_=out16[:].rearrange("p a b c -> p (a b c)"))
            nc.gpsimd.collective_compute(
                "ReduceScatter", mybir.AluOpType.add,
                replica_groups=[list(range(N_CORES))],
                ins=[rs_in[:].opt()], outs=[rs_out[:].opt()])
            nc.sync.dma_start(out=acc_d[:], in_=rs_out[:])
    nc.compile()
    return nc


def _get_program_and_slots(tabs):
    if 'prog' not in _CACHE:
        meta = build_slot_assignment(tabs)
        _CACHE['slots'] = meta
        _CACHE['prog'] = _build_program(meta)
    return _CACHE['prog'], _CACHE['slots']


def _pack_inputs(idata, qdata, tabs, wtabs, meta):
    """Per-core input dicts: quantized container rows + idx + weight tables."""
    slots = meta['slots']
    i0 = tabs['i0']

    scale = np.float32(127.0 / QCLIP)
    qi = np.clip(np.rint(idata * scale), -127, 127).astype(np.int8)
    qq = np.clip(np.rint(qdata * scale), -127, 127).astype(np.int8)
    cont = np.empty((T, E, S // 2, 4), np.int8)
    cont[..., 0] = qi[..., 0::2]
    cont[..., 1] = qq[..., 0::2]
    cont[..., 2] = qi[..., 1::2]
    cont[..., 3] = qq[..., 1::2]
    contf = cont.reshape(T, E, (S // 2) * 4).view(np.float32)  # [T,E,S//2]

    # wrapped idx: wrapped[p, s] = idx_z[s*16 + p] for p in [0,16)
    pp = np.arange(16)[:, None]

    in_maps = []
    for core in range(N_CORES):
        rows = np.zeros(meta['tot_cont'], np.float32)
        idx = np.zeros((16, meta['tot_idx']), np.int16)
        wts = np.zeros((128, meta['tot_w']), np.float16)
        for sl in slots:
            delta = sl['deltas'][core]
            if delta == DUMMY:
                continue
            ext, toff = sl['ext'], sl['toff']
            nblk, ncont, blk0 = sl['nblk'], sl['ncont'], sl['blk0']
            j = delta + 127
            c0 = sl['c0'][delta]
            if delta >= 0:
                ts = np.arange(delta, T)
            else:
                ts = np.arange(0, T + delta)
            ps = ts - toff
            width = min(ncont, S // 2 - c0)
            reg = rows[sl['coff']:sl['coff'] + ext * ncont].reshape(ext, ncont)
            reg[ps, :width] = contf[ts, ts - delta, c0:c0 + width]

            zsel = np.arange(blk0 * 128, Z)
            idx0 = np.clip((i0[j, zsel] >> 1) - c0, 0, ncont - 2)
            ss = np.arange(nblk * 8)[None, :] * 16
            wrap0 = idx0[ss + pp].astype(np.int16)          # [16, nblk*8]
            idx[:, sl['ioff']:sl['ioff'] + nblk * 8] = wrap0

            # wts[p, woff + tab*nblk + blk] = wtabs[j, tab, (blk0+blk)*128+p]
            wts[:, sl['woff']:sl['woff'] + 4 * nblk] = (
                wtabs[j, :, blk0 * 128:].reshape(4, nblk, 128)
                .transpose(2, 0, 1).reshape(128, 4 * nblk))
        in_maps.append({"rows": rows, "idx": idx, "wts": wts})
    return in_maps


def kernel(idata, qdata, grid, tx_ori, ele_pos, time_zero,
           fs, c, fdemod, rxfnum):
    from concourse.bass_utils import run_bass_kernel_spmd

    idata = _f32(idata); qdata = _f32(qdata)
    tabs = compute_tables(grid, tx_ori, ele_pos, time_zero,
                          fs, c, fdemod, rxfnum)
    wtabs = build_weight_tables(tabs)
    nc, meta = _get_program_and_slots(tabs)
    in_maps = _pack_inputs(idata, qdata, tabs, wtabs, meta)
    res = run_bass_kernel_spmd(nc, in_maps, list(range(N_CORES)))
    _CACHE['last_results'] = res

    # reassemble the reduce-scattered [128, 2, 16, 128] f16 sum
    full = np.concatenate([res.results[cidx]["acc"]
                           for cidx in range(N_CORES)], axis=0)
    full = full.astype(np.float32).reshape(128, 2, 16, 128)
    idas = full[:, 0].transpose(1, 0, 2).reshape(Z, T).T.copy()
    qdas = full[:, 1].transpose(1, 0, 2).reshape(Z, T).T.copy()
    cI, cQ = corrections(idata, qdata, tabs)
    idas += cI
    qdas += cQ
    return (idas, qdas)


# revision 25
# speedup vs baseline: 1.0310x; 1.0310x over previous
"""Trainium2 Bass kernel for DAS (delay-and-sum) ultrasound beamforming.

Math: the per-(t,e,z) delay/phase/apodization depend on (t,e) only through
vx = gx[t]-ex[e], i.e. on delta = t-e (Toeplitz geometry). Per-delta tables
(gather index, fused interp/rotation/apod weights) are computed on host from
the small geometry inputs; the sample data is processed on 8 NeuronCores.

The wall clock is dominated by host->device transfer over the axon tunnel
(~55-75 MB/s), so the packing minimizes bytes three ways:
  1. delta trim: the dynamic-aperture mask kills every delta with |delta|>=100
     (and, per delta, all z below a threshold), so only 199 of 255 diagonals
     ship, each with only its contributing z-blocks.
  2. sample-window trim: per delta only samples in [i0(zlo), i0(zmax)+1] are
     ever gathered (~1.2-1.4k of 4096), so only that window ships.
  3. int8 quantization (clip at 4 sigma, scale folded into the weight tables;
     ~0.9% rel error vs the 2e-2 budget). I/Q sample pairs are packed as 4
     int8 bytes = one f32 "container" so gpsimd ap_gather (which needs
     4-byte units) fetches I[2c],Q[2c],I[2c+1],Q[2c+1] in one d=1 lookup;
     interpolation parity (i0 even/odd) is folded into the host-built
     weight tables.

Device per (core, slot) = one delta diagonal: DMA the diagonal's container
rows -> ap_gather at container c and c+1 (indices shared across partitions =
transmits) -> extract 6 int8 byte-planes to f32 -> PE transpose to [z, t] ->
multiply by per-delta weight columns (free-axis broadcast) and accumulate.
The weight tables ship factored as (C, S, a0, a2) and are expanded on-device
(a1 = 1-a0-a2); gather indices ship once ([16, n] wrapped rows, replicated
x8 on-device; the +1 table is an on-device int16 add). The per-core partials
are summed by an on-device fp16 ReduceScatter, so each core fetches only
1/8 of the combined I|Q result; donated output buffers are zeroed on-device
instead of uploaded.

The apodization mask is validated exactly per (t,e,z) on host; any mismatch
vs the delta-representative mask is fixed with sparse host corrections
(zero for the reference geometry).

SPMD uniformity: 199 active deltas + 1 dummy = 200 (core,slot) instances in
25 slots x 8 cores, grouped by |delta| so every core's slot k has the same
compiled extents/offsets.
"""
import os
import sys

for _p in ('/opt/trn_rl_repo', '/root/.axon_site/_ro/trn_rl_repo'):
    if os.path.isdir(_p) and _p not in sys.path:
        sys.path.append(_p)

import numpy as np


def _enable_jax_compile_cache():
    # run_bass_kernel_spmd re-jits (and re-runs the BIR->NEFF pipeline) on
    # every call; the persistent cache turns that ~0.4s into a disk hit.
    try:
        import jax
        jax.config.update("jax_compilation_cache_dir", "/tmp/jaxcache")
        jax.config.update("jax_persistent_cache_min_entry_size_bytes", 0)
        jax.config.update("jax_persistent_cache_min_compile_time_secs", 0.0)
    except Exception:
        pass


_enable_jax_compile_cache()


def _install_pjrt_jit_cache():
    """Memoize the jax plumbing of bass2jax.run_bass_via_pjrt per Bass module.

    run_bass_kernel_spmd builds a fresh jit(shard_map(...)) closure on every
    call, so each call pays ~80ms of re-tracing/lowering for an identical
    computation. This caches the jitted callable per `nc`; inputs are still
    uploaded and the NEFF still executes on every call.
    """
    try:
        from concourse import bass2jax, mybir
        import jax
        from jax.sharding import Mesh, PartitionSpec
        import warnings
        with warnings.catch_warnings():
            warnings.simplefilter("ignore")
            from jax.experimental.shard_map import shard_map
    except Exception:
        return
    if getattr(bass2jax.run_bass_via_pjrt, '_das_cached', False):
        return
    orig = bass2jax.run_bass_via_pjrt
    cache = {}

    def cached_run(nc, in_maps, n_cores):
        if n_cores == 1 or nc.dbg_addr is not None:
            return orig(nc, in_maps, n_cores)
        ent = cache.get(id(nc))
        if ent is None:
            bass2jax.install_neuronx_cc_hook()
            partition_name = (nc.partition_id_tensor.name
                              if nc.partition_id_tensor else None)
            in_names, out_names, out_avals, zero_shapes = [], [], [], []
            for alloc in nc.m.functions[0].allocations:
                if not isinstance(alloc, mybir.MemoryLocationSet):
                    continue
                name = alloc.memorylocations[0].name
                if alloc.kind == "ExternalInput":
                    if name != partition_name:
                        in_names.append(name)
                elif alloc.kind == "ExternalOutput":
                    shape = tuple(alloc.tensor_shape)
                    dtype = mybir.dt.np(alloc.dtype)
                    out_avals.append(jax.core.ShapedArray(shape, dtype))
                    out_names.append(name)
                    zero_shapes.append((shape, dtype))
            n_params = len(in_names)
            all_names = in_names + out_names + (
                [partition_name] if partition_name else [])
            donate = tuple(range(n_params, n_params + len(out_names)))

            def _body(*args):
                operands = list(args)
                if partition_name is not None:
                    operands.append(bass2jax.partition_id_tensor())
                return tuple(_bind(*operands))

            def _bind(*operands):
                return bass2jax._bass_exec_p.bind(
                    *operands, out_avals=tuple(out_avals),
                    in_names=tuple(all_names), out_names=tuple(out_names),
                    lowering_input_output_aliases=(),
                    sim_require_finite=True, sim_require_nnan=True, nc=nc)

            devices = jax.devices()[:n_cores]
            mesh = Mesh(np.asarray(devices), ("core",))
            sharding = jax.sharding.NamedSharding(mesh, PartitionSpec("core"))
            nio = n_params + len(out_names)
            sharded = jax.jit(
                shard_map(_body, mesh=mesh,
                          in_specs=(PartitionSpec("core"),) * nio,
                          out_specs=(PartitionSpec("core"),) * len(out_names),
                          check_rep=False),
                donate_argnums=donate, keep_unused=True)
            ent = (in_names, out_names, out_avals, zero_shapes, sharded,
                   devices, sharding)
            cache[id(nc)] = ent
        (in_names, out_names, out_avals, zero_shapes, sharded,
         devices, sharding) = ent

        def _global(per_core):
            # assemble a sharded global array from per-device shards without
            # the concat + re-slice host memcpy round-trip
            shape = (n_cores * per_core[0].shape[0], *per_core[0].shape[1:])
            shards = [jax.device_put(a, dev)
                      for a, dev in zip(per_core, devices)]
            return jax.make_array_from_single_device_arrays(
                shape, sharding, shards)

        global_in = [_global([np.asarray(m[name]) for m in in_maps])
                     for name in in_names]
        # donated output buffers: allocate+zero on device, skip the upload
        zkey = (id(nc), 'zeros')
        zfn = cache.get(zkey)
        if zfn is None:
            import jax.numpy as jnp
            zfn = jax.jit(
                lambda: tuple(
                    jnp.zeros((n_cores * shape[0], *shape[1:]), dtype)
                    for (shape, dtype) in zero_shapes),
                out_shardings=(sharding,) * len(zero_shapes))
            cache[zkey] = zfn
        out_arrs = sharded(*global_in, *zfn())
        return [
            {name: np.asarray(out_arrs[i]).reshape(
                n_cores, *out_avals[i].shape)[c]
             for i, name in enumerate(out_names)}
            for c in range(n_cores)]

    cached_run._das_cached = True
    bass2jax.run_bass_via_pjrt = cached_run


_install_pjrt_jit_cache()

T, E, S, Z = 128, 128, 4096, 2048
PI = 3.14159265359
MIN_WIDTH = 0.001
N_CORES = 8
DUMMY = 999
QCLIP = 4.0
QSCALE = np.float32(QCLIP / 127.0)   # dequant scale, folded into weights


def _f32(x):
    return np.asarray(x, dtype=np.float32)


# ---------------------------------------------------------------- host math
def compute_tables(grid, tx_ori, ele_pos, time_zero, fs, c, fdemod, rxfnum):
    grid = _f32(grid); tx_ori = _f32(tx_ori); ele_pos = _f32(ele_pos)
    time_zero = _f32(time_zero)
    gx = grid[:, 0, 0]
    zax = grid[0, :, 2]
    ex = ele_pos[:, 0]

    vx_te = (gx[:, None] - ex[None, :]).astype(np.float32)
    vz = zax.astype(np.float32)
    with np.errstate(divide='ignore', invalid='ignore'):
        ratio = np.abs(vz[None, None, :] / vx_te[:, :, None])
    m = ratio > np.float32(rxfnum)
    m |= (np.abs(vx_te) <= np.float32(MIN_WIDTH))[:, :, None]
    m |= ((vx_te >= np.float32(MIN_WIDTH)) & (gx[:, None] <= ex[0]))[:, :, None]
    m |= ((vx_te <= np.float32(-MIN_WIDTH)) & (gx[:, None] >= ex[-1]))[:, :, None]
    mask_exact = m

    d3 = grid - tx_ori[:, None, :]
    txdel = np.sqrt((d3 * d3).sum(-1, dtype=np.float32)).astype(np.float32)

    nd = 255
    i0_tab = np.zeros((nd, Z), np.int32)
    frac_tab = np.zeros((nd, Z), np.float32)
    ct_tab = np.zeros((nd, Z), np.float32)
    st_tab = np.zeros((nd, Z), np.float32)
    v0_tab = np.zeros((nd, Z), np.float32)
    v1_tab = np.zeros((nd, Z), np.float32)
    mask_tab = np.zeros((nd, Z), bool)
    for delta in range(-127, 128):
        t_rep = max(0, delta); e_rep = t_rep - delta
        vx = vx_te[t_rep, e_rep]
        rx = np.sqrt(vx * vx + vz * vz).astype(np.float32)
        delays = ((txdel[t_rep] + rx) / np.float32(c)
                  + time_zero[t_rep]) * np.float32(fs)
        i0f = np.floor(delays)
        frac = (delays - i0f).astype(np.float32)
        i0 = i0f.astype(np.int32)
        tshift = delays / np.float32(fs) - zax * np.float32(2.0) / np.float32(c)
        theta = (np.float32(2.0 * PI * fdemod) * tshift).astype(np.float32)
        j = delta + 127
        i0_tab[j] = i0
        frac_tab[j] = frac
        ct_tab[j] = np.cos(theta, dtype=np.float32)
        st_tab[j] = np.sin(theta, dtype=np.float32)
        v0_tab[j] = (i0 >= 0) & (i0 < S)
        v1_tab[j] = (i0 + 1 >= 0) & (i0 + 1 < S)
        mask_tab[j] = mask_exact[t_rep, e_rep]
    return dict(i0=i0_tab, frac=frac_tab, ct=ct_tab, st=st_tab,
                v0=v0_tab, v1=v1_tab, mask_tab=mask_tab,
                mask_exact=mask_exact)


def build_slot_assignment(tabs):
    """25 slots x 8 cores over the mask-active deltas, grouped by |delta|.

    Per slot (uniform across cores, compile-time): toff/ext (transmit rows),
    blk0/nblk (contributing z-blocks of 128), ncont (container window width).
    Per (slot, core): the delta, and its container base c0.
    """
    i0 = tabs['i0']; mask = tabs['mask_tab']
    active = sorted(d for d in range(-127, 128) if mask[d + 127].any())
    dmax = max(abs(d) for d in active)
    assert all(abs(d) <= dmax for d in active)

    zlo = {}
    c0 = {}
    cmax = {}
    for d in active:
        j = d + 127
        nz = np.nonzero(mask[j])[0]
        zlo[d] = int(nz[0])
        assert mask[j, zlo[d]:].all(), "mask must be a suffix in z"
        assert (np.diff(i0[j]) >= 0).all(), "i0 must be monotone in z"
        lo = int(i0[j, zlo[d]]); hi = int(i0[j, Z - 1])
        assert lo >= 0 and hi + 1 < S, "window must be interior"
        c0[d] = lo >> 1
        cmax[d] = (hi >> 1) + 1          # container of highest sample, +1 pad

    # groups: positive runs of 8 ascending from 4, negative mirror, plus the
    # near-diagonal mixed group {0,+-1,+-2,+-3} + dummy
    small = [d for d in active if abs(d) <= 3]
    pos = sorted(d for d in active if d > 3)
    neg = sorted((d for d in active if d < -3), key=lambda d: -d)
    assert len(small) == 7 and len(pos) % 8 == 0 and len(neg) % 8 == 0

    groups = [small + [DUMMY]]
    for i in range(0, len(pos), 8):
        groups.append(pos[i:i + 8])
    for i in range(0, len(neg), 8):
        groups.append(neg[i:i + 8])

    slots = []
    coff = ioff = woff = 0
    for g in groups:
        real = [d for d in g if d != DUMMY]
        toff = min(max(0, d) for d in real)
        ext = 128 - min(abs(d) for d in real)
        blk0 = min(zlo[d] for d in real) // 128
        nblk = 16 - blk0
        ncont = max(cmax[d] - c0[d] + 1 for d in real)
        slots.append(dict(deltas=list(g), toff=toff, ext=ext, blk0=blk0,
                          nblk=nblk, ncont=ncont,
                          c0={d: c0[d] for d in real},
                          coff=coff, ioff=ioff, woff=woff))
        coff += ext * ncont
        ioff += nblk * 8
        woff += 4 * nblk
    return dict(slots=slots, tot_cont=coff, tot_idx=ioff, tot_w=woff)


def build_weight_tables(tabs):
    """Per-delta z-tables for the 6 gathered byte planes.

    Gather0 fetches container c=(i0>>1)-c0: bytes (I[2c],Q[2c],I[2c+1],Q[2c+1]);
    gather1 fetches container c+1. With parity p = i0&1:
      I[i0]   = p==0 ? g0.b0 : g0.b2      I[i0+1] = p==0 ? g0.b2 : g1.b0
    Plane coefficients a = (a_b0, a_b2, a_g1b0):
      a_b0 = even*(1-frac); a_b2 = even*frac + odd*(1-frac); a_g1b0 = odd*frac
    Shipped tables (x apod x quant scale): C = ct, S = st, plus a0 and a2;
    the device expands u_j = C*a_j, v_j = S*a_j with a1 = 1 - a0 - a2.
      accI = sum_j u_j*Iplane_j - v_j*Qplane_j
      accQ = sum_j v_j*Iplane_j + u_j*Qplane_j
    """
    apod = tabs['mask_tab'].astype(np.float32) * QSCALE
    frac = tabs['frac']
    even = ((tabs['i0'] & 1) == 0).astype(np.float32)
    odd = np.float32(1.0) - even
    out = np.empty((255, 4, Z), np.float32)
    out[:, 0] = apod * tabs['ct']
    out[:, 1] = apod * tabs['st']
    out[:, 2] = even * (np.float32(1.0) - frac)
    out[:, 3] = odd * frac
    return out


def corrections(idata, qdata, tabs):
    corrI = np.zeros((T, Z), np.float32)
    corrQ = np.zeros((T, Z), np.float32)
    i0c = np.clip(tabs['i0'], 0, S - 1)
    i1c = np.clip(tabs['i0'] + 1, 0, S - 1)
    for delta in range(-127, 128):
        j = delta + 127
        ts = np.arange(max(0, delta), min(T - 1, T - 1 + delta) + 1)
        es = ts - delta
        dm = (tabs['mask_exact'][ts, es, :].astype(np.int8)
              - tabs['mask_tab'][j][None, :].astype(np.int8))
        nz = np.argwhere(dm != 0)
        if nz.size == 0:
            continue
        ti, zi = nz[:, 0], nz[:, 1]
        tt, ee = ts[ti], es[ti]
        sgn = dm[ti, zi].astype(np.float32)
        f = tabs['frac'][j][zi]; ct = tabs['ct'][j][zi]; st = tabs['st'][j][zi]
        v0 = tabs['v0'][j][zi]; v1 = tabs['v1'][j][zi]
        I0 = idata[tt, ee, i0c[j][zi]] * v0; I1 = idata[tt, ee, i1c[j][zi]] * v1
        Q0 = qdata[tt, ee, i0c[j][zi]] * v0; Q1 = qdata[tt, ee, i1c[j][zi]] * v1
        fi = (1 - f) * I0 + f * I1
        fq = (1 - f) * Q0 + f * Q1
        np.add.at(corrI, (tt, zi), sgn * (ct * fi - st * fq))
        np.add.at(corrQ, (tt, zi), sgn * (ct * fq + st * fi))
    return corrI, corrQ


# ------------------------------------------------------------- bass program
_CACHE = {}


def _build_program(meta):
    import concourse.bacc as bacc
    import concourse.mybir as mybir
    from concourse.tile import TileContext
    from concourse.masks import make_identity

    F32 = mybir.dt.float32
    F16 = mybir.dt.float16
    I16 = mybir.dt.int16
    I8 = mybir.dt.int8
    slots = meta['slots']

    nc = bacc.Bacc("TRN2", target_bir_lowering=False, debug=False,
                   num_devices=N_CORES)
    rows_d = nc.dram_tensor("rows", [meta['tot_cont']], F32,
                            kind="ExternalInput").ap()
    # wrapped gather indices repeat identically across the 8 16-partition
    # groups, so only 16 partitions ship; a broadcast DMA replicates x8.
    idx_d = nc.dram_tensor("idx", [16, meta['tot_idx']], I16,
                           kind="ExternalInput").ap()
    wts_d = nc.dram_tensor("wts", [128, meta['tot_w']], F16,
                           kind="ExternalInput").ap()
    # combined I|Q partial sums, reduce-scattered across the 8 cores:
    # core c outputs partitions [16c, 16c+16) of the summed [128, 2Z].
    acc_d = nc.dram_tensor("acc", [16, 2 * Z], F16, kind="ExternalOutput").ap()

    with TileContext(nc) as tc:
        with tc.tile_pool(name="data", bufs=2) as dpool, \
             tc.tile_pool(name="gout", bufs=2) as gpool, \
             tc.tile_pool(name="plane", bufs=2) as plpool, \
             tc.tile_pool(name="small", bufs=2) as spool, \
             tc.tile_pool(name="tmp", bufs=3) as tpool, \
             tc.tile_pool(name="accp", bufs=1) as apool, \
             tc.tile_pool(name="dram", bufs=1, space="DRAM") as drpool, \
             tc.tile_pool(name="psum", bufs=2, space="PSUM") as ppool:
            ident = apool.tile([128, 128], F32, tag="ident")
            make_identity(nc, ident[:])
            accI = apool.tile([128, 16, 128], F32, tag="accI")
            accQ = apool.tile([128, 16, 128], F32, tag="accQ")
            nc.vector.memset(accI[:], 0.0)
            nc.vector.memset(accQ[:], 0.0)

            for k, sl in enumerate(slots):
                ext, toff = sl['ext'], sl['toff']
                nblk, ncont = sl['nblk'], sl['ncont']
                nidx = nblk * 128
                data_t = dpool.tile([128, ncont], F32, tag="data")
                nc.sync.dma_start(
                    out=data_t[0:ext],
                    in_=rows_d[sl['coff']:sl['coff'] + ext * ncont]
                    .rearrange("(r c) -> r c", c=ncont))
                idx_t = spool.tile([128, nblk * 8], I16, tag="idx")
                for g in range(8):
                    nc.sync.dma_start(
                        out=idx_t[16 * g:16 * g + 16, :],
                        in_=idx_d[:, sl['ioff']:sl['ioff'] + nblk * 8])
                idx1_t = spool.tile([128, nblk * 8], I16, tag="idx1")
                nc.any.tensor_scalar(out=idx1_t[:], in0=idx_t[:], scalar1=1,
                                     scalar2=None, op0=mybir.AluOpType.add)
                w_t = spool.tile([128, 4 * nblk], F16, tag="wts")
                nc.sync.dma_start(
                    out=w_t[:], in_=wts_d[:, sl['woff']:sl['woff'] + 4 * nblk])
                # expand (C, S, a0, a2) -> [u0|u1|u2|v0|v1|v2], a1 = 1-a0-a2
                wf = spool.tile([128, 6 * nblk], F16, tag="wfull")
                C, Sw = w_t[:, 0:nblk], w_t[:, nblk:2 * nblk]
                A0, A2 = w_t[:, 2 * nblk:3 * nblk], w_t[:, 3 * nblk:4 * nblk]
                for base, src in ((0, C), (3, Sw)):
                    s0 = wf[:, base * nblk:(base + 1) * nblk]
                    s1 = wf[:, (base + 1) * nblk:(base + 2) * nblk]
                    s2 = wf[:, (base + 2) * nblk:(base + 3) * nblk]
                    nc.any.tensor_tensor(out=s0, in0=src, in1=A0,
                                         op=mybir.AluOpType.mult)
                    nc.any.tensor_tensor(out=s2, in0=src, in1=A2,
                                         op=mybir.AluOpType.mult)
                    nc.any.tensor_tensor(out=s1, in0=src, in1=s0,
                                         op=mybir.AluOpType.subtract)
                    nc.any.tensor_tensor(out=s1, in0=s1, in1=s2,
                                         op=mybir.AluOpType.subtract)

                gout0 = gpool.tile([128, nidx], F32, tag="g0")
                gout1 = gpool.tile([128, nidx], F32, tag="g1")
                ch = min(128, (ext + 15) // 16 * 16)
                nc.gpsimd.ap_gather(gout0[0:ch], data_t[0:ch], idx_t[:],
                                    channels=ch, num_elems=ncont, d=1,
                                    num_idxs=nidx)
                nc.gpsimd.ap_gather(gout1[0:ch], data_t[0:ch], idx1_t[:],
                                    channels=ch, num_elems=ncont, d=1,
                                    num_idxs=nidx)
                g0b = gout0[:].bitcast(I8).rearrange("p (n b) -> p n b", b=4)
                g1b = gout1[:].bitcast(I8).rearrange("p (n b) -> p n b", b=4)

                # (byte-plane source, weight-table index j, is_I_plane)
                for (src, j, isI) in ((g0b[:, :, 0], 0, True),
                                      (g0b[:, :, 2], 1, True),
                                      (g1b[:, :, 0], 2, True),
                                      (g0b[:, :, 1], 0, False),
                                      (g0b[:, :, 3], 1, False),
                                      (g1b[:, :, 1], 2, False)):
                    plane = plpool.tile([128, nidx], F32, tag="plane")
                    nc.any.tensor_copy(out=plane[0:ch], in_=src[0:ch])
                    big = ppool.tile([128, nblk, 128], F32, space="PSUM",
                                     tag="big")
                    for blk in range(nblk):
                        nc.tensor.transpose(
                            out=big[:, blk, 0:ch],
                            in_=plane[0:ch, blk * 128:(blk + 1) * 128],
                            identity=ident[0:ch, 0:ch])
                    u = wf[:, j * nblk:(j + 1) * nblk]
                    v = wf[:, (3 + j) * nblk:(4 + j) * nblk]
                    pairs = (((accI, u, mybir.AluOpType.add),
                              (accQ, v, mybir.AluOpType.add)) if isI else
                             ((accI, v, mybir.AluOpType.subtract),
                              (accQ, u, mybir.AluOpType.add)))
                    for (acc, w_ap, op) in pairs:
                        tmp = tpool.tile([128, nblk, 128], F32, tag="tmp")
                        nc.any.tensor_tensor(
                            out=tmp[:, :, 0:ext], in0=big[:, :, 0:ext],
                            in1=w_ap.broadcast_to([128, nblk, ext]),
                            op=mybir.AluOpType.mult)
                        acc_ap = acc[:, sl['blk0']:sl['blk0'] + nblk,
                                     toff:toff + ext]
                        nc.any.tensor_tensor(out=acc_ap, in0=acc_ap,
                                             in1=tmp[:, :, 0:ext], op=op)

            out16 = apool.tile([128, 2, 16, 128], F16, tag="o16")
            nc.any.tensor_copy(out=out16[:, 0], in_=accI[:])
            nc.any.tensor_copy(out=out16[:, 1], in_=accQ[:])
            rs_in = drpool.tile([128, 2 * Z], F16, tag="rs_in")
            rs_out = drpool.tile([16, 2 * Z], F16, tag="rs_out")
            nc.sync.dma_start(out=rs_in[:],
                              in_=out16[:].rearrange("p a b c -> p (a b c)"))
            nc.gpsimd.collective_compute(
                "ReduceScatter", mybir.AluOpType.add,
                replica_groups=[list(range(N_CORES))],
                ins=[rs_in[:].opt()], outs=[rs_out[:].opt()])
            nc.sync.dma_start(out=acc_d[:], in_=rs_out[:])
    nc.compile()
    return nc


def _get_program_and_slots(tabs):
    if 'prog' not in _CACHE:
        meta = build_slot_assignment(tabs)
        _CACHE['slots'] = meta
        _CACHE['prog'] = _build_program(meta)
    return _CACHE['prog'], _CACHE['slots']


def _pack_inputs(idata, qdata, tabs, wtabs, meta):
    """Per-core input dicts: quantized container rows + idx + weight tables."""
    slots = meta['slots']
    i0 = tabs['i0']

    scale = np.float32(127.0 / QCLIP)
    qi = np.clip(np.rint(idata * scale), -127, 127).astype(np.int8)
    qq = np.clip(np.rint(qdata * scale), -127, 127).astype(np.int8)
    cont = np.empty((T, E, S // 2, 4), np.int8)
    cont[..., 0] = qi[..., 0::2]
    cont[..., 1] = qq[..., 0::2]
    cont[..., 2] = qi[..., 1::2]
    cont[..., 3] = qq[..., 1::2]
    contf = cont.reshape(T, E, (S // 2) * 4).view(np.float32)  # [T,E,S//2]

    # wrapped idx: wrapped[p, s] = idx_z[s*16 + p] for p in [0,16)
    pp = np.arange(16)[:, None]

    in_maps = []
    for core in range(N_CORES):
        rows = np.zeros(meta['tot_cont'], np.float32)
        idx = np.zeros((16, meta['tot_idx']), np.int16)
        wts = np.zeros((128, meta['tot_w']), np.float16)
        for sl in slots:
            delta = sl['deltas'][core]
            if delta == DUMMY:
                continue
            ext, toff = sl['ext'], sl['toff']
            nblk, ncont, blk0 = sl['nblk'], sl['ncont'], sl['blk0']
            j = delta + 127
            c0 = sl['c0'][delta]
            if delta >= 0:
                ts = np.arange(delta, T)
            else:
                ts = np.arange(0, T + delta)
            ps = ts - toff
            width = min(ncont, S // 2 - c0)
            reg = rows[sl['coff']:sl['coff'] + ext * ncont].reshape(ext, ncont)
            reg[ps, :width] = contf[ts, ts - delta, c0:c0 + width]

            zsel = np.arange(blk0 * 128, Z)
            idx0 = np.clip((i0[j, zsel] >> 1) - c0, 0, ncont - 2)
            ss = np.arange(nblk * 8)[None, :] * 16
            wrap0 = idx0[ss + pp].astype(np.int16)          # [16, nblk*8]
            idx[:, sl['ioff']:sl['ioff'] + nblk * 8] = wrap0

            # wts[p, woff + tab*nblk + blk] = wtabs[j, tab, (blk0+blk)*128+p]
            wts[:, sl['woff']:sl['woff'] + 4 * nblk] = (
                wtabs[j, :, blk0 * 128:].reshape(4, nblk, 128)
                .transpose(2, 0, 1).reshape(128, 4 * nblk))
        in_maps.append({"rows": rows, "idx": idx, "wts": wts})
    return in_maps


def kernel(idata, qdata, grid, tx_ori, ele_pos, time_zero,
           fs, c, fdemod, rxfnum):
    from concourse.bass_utils import run_bass_kernel_spmd

    idata = _f32(idata); qdata = _f32(qdata)
    tabs = compute_tables(grid, tx_ori, ele_pos, time_zero,
                          fs, c, fdemod, rxfnum)
    wtabs = build_weight_tables(tabs)
    nc, meta = _get_program_and_slots(tabs)
    in_maps = _pack_inputs(idata, qdata, tabs, wtabs, meta)
    res = run_bass_kernel_spmd(nc, in_maps, list(range(N_CORES)))
    _CACHE['last_results'] = res

    # reassemble the reduce-scattered [128, 2, 16, 128] f16 sum
    full = np.concatenate([res.results[cidx]["acc"]
                           for cidx in range(N_CORES)], axis=0)
    full = full.astype(np.float32).reshape(128, 2, 16, 128)
    idas = full[:, 0].transpose(1, 0, 2).reshape(Z, T).T.copy()
    qdas = full[:, 1].transpose(1, 0, 2).reshape(Z, T).T.copy()
    cI, cQ = corrections(idata, qdata, tabs)
    idas += cI
    qdas += cQ
    return (idas, qdas)
